# revision 7
# baseline (speedup 1.0000x reference)
"""Trainium2 Bass kernel for a 2-layer GRU time-series binary classifier.

Model (torch GRU semantics, batch_first):
  seq1, _ = GRU(F=2048 -> H1=128)(x)        x: [64, 512, 2048]
  _,  h2 = GRU(H1 -> H2=64)(seq1)
  out = h2 @ fc_w.T + fc_b                  -> [64, 1]

Strategy: data-parallel over batch across 8 cores (8 sequences each).
Per core, layer-1's input projection runs as a chunked bf16 GEMM whose
gate outputs stay in PSUM; the recurrent h @ W_hh.T matmuls accumulate
into the same PSUM banks (start=False on set has_written bits), so the
r/z gates need no explicit adds.  State h lives in [H, B] layout so no
transposes appear anywhere.  Layer 2 runs one 32-step chunk behind
layer 1; its input projection consumes layer-1's h history directly.
"""

import numpy as np
import ml_dtypes

from concourse import bacc, tile, mybir
from concourse.bass_utils import run_bass_kernel_spmd

BF16 = ml_dtypes.bfloat16
N_CORES = 8
B, T, F = 64, 512, 2048
H1, H2 = 128, 64
B_LOC = B // N_CORES          # 8 sequences per core
CHUNK = 32                    # timesteps per GEMM chunk
NCH = T // CHUNK              # 16 chunks
AF = mybir.ActivationFunctionType
ALU = mybir.AluOpType
DT_BF = mybir.dt.bfloat16
DT_F32 = mybir.dt.float32


def build_nc():
    nc = bacc.Bacc(None, target_bir_lowering=False)

    xT = nc.declare_dram_parameter("xT", [F, T, B_LOC], DT_BF, isOutput=False)
    wih1T = nc.declare_dram_parameter("wih1T", [F, 3 * H1], DT_BF, isOutput=False)
    whh1T = nc.declare_dram_parameter("whh1T", [H1, 3 * H1], DT_BF, isOutput=False)
    wih2T = nc.declare_dram_parameter("wih2T", [H1, 3 * H2], DT_BF, isOutput=False)
    whh2T = nc.declare_dram_parameter("whh2T", [H2, 3 * H2], DT_BF, isOutput=False)
    brow1 = nc.declare_dram_parameter("brow1", [1, 3 * H1], DT_BF, isOutput=False)
    brow2 = nc.declare_dram_parameter("brow2", [1, 3 * H2], DT_BF, isOutput=False)
    bhn1 = nc.declare_dram_parameter("bhn1", [H1, 1], DT_F32, isOutput=False)
    bhn2 = nc.declare_dram_parameter("bhn2", [H2, 1], DT_F32, isOutput=False)
    fcwT = nc.declare_dram_parameter("fcwT", [H2, 1], DT_F32, isOutput=False)
    fcb = nc.declare_dram_parameter("fcb", [B_LOC, 1], DT_F32, isOutput=False)
    out = nc.declare_dram_parameter("out", [B_LOC, 1], DT_F32, isOutput=True)

    KT = F // 128              # 16 K-tiles for GEMM1
    NW = CHUNK * B_LOC         # 256 moving columns per chunk GEMM

    with tile.TileContext(nc) as tc:
        with (
            tc.tile_pool(name="const", bufs=1) as cpool,
            tc.tile_pool(name="xchunk", bufs=3) as xpool,
            tc.tile_pool(name="xn", bufs=2) as xnpool,
            tc.tile_pool(name="step", bufs=3) as spool,
            tc.tile_pool(name="psum", bufs=2, space="PSUM") as ppool,
        ):
            # ---- persistent tiles -------------------------------------
            w1 = cpool.tile([128, KT, 3 * H1], DT_BF)      # GEMM1 stationaries
            wh1 = cpool.tile([H1, 3 * H1], DT_BF)
            w2 = cpool.tile([H1, 3 * H2], DT_BF)
            wh2 = cpool.tile([H2, 3 * H2], DT_BF)
            br1 = cpool.tile([1, 3 * H1], DT_BF)
            br2 = cpool.tile([1, 3 * H2], DT_BF)
            bn1 = cpool.tile([H1, 1], DT_F32)
            bn2 = cpool.tile([H2, 1], DT_F32)
            fw = cpool.tile([H2, 1], DT_F32)
            fb = cpool.tile([B_LOC, 1], DT_F32)
            ones = cpool.tile([1, NW], DT_BF)
            h1h = cpool.tile([H1, (T + 1) * B_LOC], DT_BF)  # h1 history
            h2h = cpool.tile([H2, (T + 1) * B_LOC], DT_BF)
            h2fin = cpool.tile([H2, B_LOC], DT_F32)

            nc.sync.dma_start(out=w1[:], in_=wih1T.rearrange("(kt p) g -> p kt g", p=128))
            nc.sync.dma_start(out=wh1[:], in_=whh1T[:])
            nc.sync.dma_start(out=w2[:], in_=wih2T[:])
            nc.sync.dma_start(out=wh2[:], in_=whh2T[:])
            nc.sync.dma_start(out=br1[:], in_=brow1[:])
            nc.sync.dma_start(out=br2[:], in_=brow2[:])
            nc.sync.dma_start(out=bn1[:], in_=bhn1[:])
            nc.sync.dma_start(out=bn2[:], in_=bhn2[:])
            nc.sync.dma_start(out=fw[:], in_=fcwT[:])
            nc.sync.dma_start(out=fb[:], in_=fcb[:])
            nc.vector.memset(ones[:], 1.0)
            nc.vector.memset(h1h[:, 0:B_LOC], 0.0)
            nc.vector.memset(h2h[:, 0:B_LOC], 0.0)

            # ---- chunk-state carried across waves ---------------------
            xtiles = {}       # chunk -> x SBUF tile [128, KT, NW]
            rz1_ps = {}       # chunk -> psum [128, 512]: r | z
            n1_ps = {}        # chunk -> psum [128, 512]: xn gemm | step hn
            rz2_ps = {}
            n2_ps = {}
            xn1_sb = {}
            xn2_sb = {}

            def dma_xchunk(c):
                xt = xpool.tile([128, KT, NW], DT_BF, tag="xc")
                nc.sync.dma_start(
                    out=xt[:],
                    in_=xT[:, c * CHUNK:(c + 1) * CHUNK, :].rearrange(
                        "(kt p) t b -> p kt (t b)", p=128),
                )
                xtiles[c] = xt

            def gemm1_closures(c):
                """Emission thunks for layer-1 input projection of chunk c."""
                rz = ppool.tile([128, 512], DT_F32, tag="l1rz")
                np_ = ppool.tile([128, 512], DT_F32, tag="l1n")
                rz1_ps[c], n1_ps[c] = rz, np_
                xt = xtiles[c]
                thunks = []
                for g, (dst, lo) in enumerate(
                    [(rz, 0), (rz, 256), (np_, 0)]):  # r, z, n
                    # start=True clears has_written for the WHOLE bank, so
                    # only the first matmul touching each bank may set it
                    # (z rides on r's clear; per-element bits handle the rest).
                    def mk(kt, g=g, dst=dst, lo=lo):
                        def f():
                            nc.tensor.matmul(
                                dst[:, lo:lo + NW],
                                w1[:, kt, g * 128:(g + 1) * 128],
                                xt[:, kt],
                                start=(kt == 0 and lo == 0), stop=False,
                                skip_group_check=True)
                        return f
                    for kt in range(KT):
                        thunks.append(mk(kt))

                    def fbias(g=g, dst=dst, lo=lo):
                        nc.tensor.matmul(
                            dst[:, lo:lo + NW],
                            br1[:, g * 128:(g + 1) * 128],
                            ones[:],
                            start=False, stop=True,
                            skip_group_check=True)
                    thunks.append(fbias)

                xs = xnpool.tile([128, NW], DT_F32, tag="xn1")
                xn1_sb[c] = xs

                def fdrain():
                    nc.scalar.copy(xs[:], np_[:, 0:NW])
                thunks.append(fdrain)
                return thunks

            def gemm2_closures(c):
                """Layer-2 input projection of chunk c (reads h1 history)."""
                rz = ppool.tile([H2, 512], DT_F32, tag="l2rz")
                np_ = ppool.tile([H2, 512], DT_F32, tag="l2n")
                rz2_ps[c], n2_ps[c] = rz, np_
                mv = h1h[:, (c * CHUNK + 1) * B_LOC:(c * CHUNK + 1 + CHUNK) * B_LOC]
                thunks = []
                for g, (dst, lo) in enumerate(
                    [(rz, 0), (rz, 256), (np_, 0)]):
                    def fmm(g=g, dst=dst, lo=lo):
                        nc.tensor.matmul(
                            dst[:, lo:lo + NW],
                            w2[:, g * H2:(g + 1) * H2],
                            mv,
                            start=(lo == 0), stop=False,
                            skip_group_check=True)
                    thunks.append(fmm)

                    def fbias(g=g, dst=dst, lo=lo):
                        nc.tensor.matmul(
                            dst[:, lo:lo + NW],
                            br2[:, g * H2:(g + 1) * H2],
                            ones[:],
                            start=False, stop=True,
                            skip_group_check=True)
                    thunks.append(fbias)

                xs = xnpool.tile([H2, NW], DT_F32, tag="xn2")
                xn2_sb[c] = xs

                def fdrain():
                    nc.scalar.copy(xs[:], np_[:, 0:NW])
                thunks.append(fdrain)
                return thunks

            def step(layer, c, t):
                """One GRU cell update, [H, B] layout."""
                if layer == 1:
                    H, hh, wh, bn, rzp, npp, xns = (
                        H1, h1h, wh1, bn1, rz1_ps[c], n1_ps[c], xn1_sb[c])
                else:
                    H, hh, wh, bn, rzp, npp, xns = (
                        H2, h2h, wh2, bn2, rz2_ps[c], n2_ps[c], xn2_sb[c])
                gt = c * CHUNK + t
                hp = hh[:, gt * B_LOC:(gt + 1) * B_LOC]
                co = t * B_LOC                        # column offset in chunk
                so = 256 + co                         # step region offset
                # recurrent matmuls: r,z accumulate onto GEMM psum; n fresh
                nc.tensor.matmul(rzp[:, co:co + B_LOC], wh[:, 0:H], hp,
                                 start=False, stop=True, skip_group_check=True)
                nc.tensor.matmul(rzp[:, 256 + co:256 + co + B_LOC],
                                 wh[:, H:2 * H], hp,
                                 start=False, stop=True, skip_group_check=True)
                nc.tensor.matmul(npp[:, so:so + B_LOC], wh[:, 2 * H:3 * H], hp,
                                 start=True, stop=True, skip_group_check=True)
                # sigmoid over r|z in one ACT op (strided 2-range AP)
                rzv = rzp.rearrange("p (g x) -> p g x", g=2)[:, :, co:co + B_LOC]
                rz_t = spool.tile([H, 2, B_LOC], DT_F32, tag=f"rz{layer}")
                nc.scalar.activation(rz_t[:], rzv, AF.Sigmoid)
                r = rz_t[:, 0]
                z = rz_t[:, 1]
                # n = tanh(xn + r * (hn + b_hhn))
                tn = spool.tile([H, B_LOC], DT_F32, tag=f"tn{layer}")
                nc.vector.scalar_tensor_tensor(
                    out=tn[:], in0=npp[:, so:so + B_LOC], scalar=bn[:],
                    in1=r, op0=ALU.add, op1=ALU.mult)
                t2 = spool.tile([H, B_LOC], DT_F32, tag=f"t2{layer}")
                nc.vector.tensor_tensor(out=t2[:], in0=tn[:],
                                        in1=xns[:, co:co + B_LOC], op=ALU.add)
                n_t = spool.tile([H, B_LOC], DT_F32, tag=f"n{layer}")
                nc.scalar.activation(n_t[:], t2[:], AF.Tanh)
                # h' = n + z*(h - n)
                d = spool.tile([H, B_LOC], DT_F32, tag=f"d{layer}")
                nc.vector.tensor_tensor(out=d[:], in0=hp, in1=n_t[:],
                                        op=ALU.subtract)
                e = spool.tile([H, B_LOC], DT_F32, tag=f"e{layer}")
                nc.vector.tensor_tensor(out=e[:], in0=rz_t[:, 1], in1=d[:],
                                        op=ALU.mult)
                if layer == 2 and gt == T - 1:
                    nc.vector.tensor_tensor(out=h2fin[:], in0=n_t[:], in1=e[:],
                                            op=ALU.add)
                else:
                    nc.vector.tensor_tensor(
                        out=hh[:, (gt + 1) * B_LOC:(gt + 2) * B_LOC],
                        in0=n_t[:], in1=e[:], op=ALU.add)

            # ---- prologue --------------------------------------------
            dma_xchunk(0)
            dma_xchunk(1)
            for f in gemm1_closures(0):
                f()

            # ---- flat slot timeline ----------------------------------
            # L1 runs step s at slot s; L2 runs step s-LAG at slot s.
            # At each chunk boundary 32k we enqueue GEMM2(k-1) (h1 chunk
            # k-1 just finished) and GEMM1(k+1); thunks pop a few per
            # slot so the PE never stalls on a GEMM block and every
            # psum is fully written before its first consumer is traced.
            LAG = CHUNK + 8
            thunks = []
            for s in range(T + LAG):
                if s % CHUNK == 0:
                    k = s // CHUNK
                    if 1 <= k <= NCH:
                        thunks += gemm2_closures(k - 1)
                    if 1 <= k + 1 < NCH:
                        thunks += gemm1_closures(k + 1)
                    if k + 2 < NCH:
                        dma_xchunk(k + 2)
                if s < T:
                    step(1, s // CHUNK, s % CHUNK)
                u = s - LAG
                if 0 <= u < T:
                    step(2, u // CHUNK, u % CHUNK)
                for _ in range(3):
                    if thunks:
                        thunks.pop(0)()
            while thunks:
                thunks.pop(0)()

            # ---- fc head ---------------------------------------------
            fcp = ppool.tile([B_LOC, 1], DT_F32, tag="l2rz")
            nc.tensor.matmul(fcp[:], h2fin[:], fw[:], start=True, stop=True,
                             skip_group_check=True)
            res = cpool.tile([B_LOC, 1], DT_F32)
            nc.scalar.activation(res[:], fcp[:], AF.Identity, bias=fb[:])
            nc.sync.dma_start(out=out[:], in_=res[:])

    nc.compile()
    return nc


_NC_CACHE = {}


def _get_nc():
    if "nc" not in _NC_CACHE:
        _NC_CACHE["nc"] = build_nc()
    return _NC_CACHE["nc"]


def _prep_maps(x, w_ih1, w_hh1, b_ih1, b_hh1, w_ih2, w_hh2, b_ih2, b_hh2,
               fc_w, fc_b):
    f32 = np.float32
    brow1 = np.concatenate([
        (b_ih1[:H1] + b_hh1[:H1]),
        (b_ih1[H1:2 * H1] + b_hh1[H1:2 * H1]),
        b_ih1[2 * H1:],
    ]).reshape(1, 3 * H1)
    brow2 = np.concatenate([
        (b_ih2[:H2] + b_hh2[:H2]),
        (b_ih2[H2:2 * H2] + b_hh2[H2:2 * H2]),
        b_ih2[2 * H2:],
    ]).reshape(1, 3 * H2)
    shared = {
        "wih1T": np.ascontiguousarray(w_ih1.T).astype(BF16),
        "whh1T": np.ascontiguousarray(w_hh1.T).astype(BF16),
        "wih2T": np.ascontiguousarray(w_ih2.T).astype(BF16),
        "whh2T": np.ascontiguousarray(w_hh2.T).astype(BF16),
        "brow1": brow1.astype(BF16),
        "brow2": brow2.astype(BF16),
        "bhn1": np.ascontiguousarray(b_hh1[2 * H1:].reshape(H1, 1), dtype=f32),
        "bhn2": np.ascontiguousarray(b_hh2[2 * H2:].reshape(H2, 1), dtype=f32),
        "fcwT": np.ascontiguousarray(fc_w.reshape(1, H2).T, dtype=f32),
        "fcb": np.full((B_LOC, 1), float(fc_b.reshape(-1)[0]), dtype=f32),
    }
    maps = []
    for c in range(N_CORES):
        xc = x[c * B_LOC:(c + 1) * B_LOC]          # [B_LOC, T, F]
        xTc = np.ascontiguousarray(xc.transpose(2, 1, 0)).astype(BF16)
        maps.append({"xT": xTc, **shared})
    return maps


def run(inputs, trace=False):
    nc = _get_nc()
    maps = _prep_maps(**inputs)
    res = run_bass_kernel_spmd(nc, maps, list(range(N_CORES)), trace=trace)
    outs = [np.asarray(res.results[i]["out"], np.float32) for i in range(N_CORES)]
    full = np.concatenate(outs, axis=0)            # [64, 1]
    return full, res.exec_time_ns


def kernel(**inputs):
    inputs = {k: np.asarray(v, np.float32) for k, v in inputs.items()}
    out, _ = run(inputs, trace=False)
    return out
